# revision 17
# baseline (speedup 1.0000x reference)
"""Sharded 2-branch GAT kernel for Trainium2 (8 NeuronCores), full-I/O SPMD.

Nodes sharded 8 x 6272. Per layer: each core projects its shard
(h = h_in @ W, es/ed = h @ a), builds 512B table rows
[h bf16(128) | 1.0 | es f32 | ed f32 | pad] and AllGathers the table.
Edges (+self loops) are sharded by dst, sorted, grouped into 128-node quads,
split by src parity into two dma_gather classes (int16 idx = src>>1 over
even/odd row views, elem_step=512), padded to 128-edge tiles with dst-span
<= 25. Per tile, ed[dst] is rebuilt from an ap_gather'd 32-wide window of the
replicated ed vector dotted with a local_scatter one-hot; p = exp(leaky(es+ed))
(softmax shift is algebraically redundant); a second local_scatter builds the
p-scaled one-hot lhsT and PE matmuls accumulate [sum p*h | sum p] per quad in
PSUM. Finalize divides by the denominator, adds bias, relu, PE-transposes.
Final MLP + output index gather run sharded; host reassembles.
"""
import os
import sys
import numpy as np

sys.path.insert(0, "/opt/trn_rl_repo")
NLAYERS = int(os.environ.get("GAT_NLAYERS", "4"))
DO_EDGES = os.environ.get("GAT_EDGES", "1") == "1"
DO_AG = os.environ.get("GAT_AG", "1") == "1"
DO_OG = os.environ.get("GAT_OG", "1") == "1"

N_NODES = 50000
C = 8
SHARD = 6272
NPAD = C * SHARD            # 50176
NQUAD = SHARD // 128        # 49
NEG_SLOPE = 0.2
ELEM = 256                  # bf16 elems per table row (512 B)
ES_F32 = 65                 # f32 slot of es within a row
MAX_SPAN = 24
QPB = 2                     # quads per slab batch
MAXT = 36                   # slab capacity in tiles
SCAT_SUB = 12               # tiles per lhsT local_scatter (12*128 <= 2047)
OUT_TILES = 16


# ---------------------------------------------------------------- host baking
def _with_loops(ei):
    loops = np.arange(N_NODES, dtype=np.int64)
    return (np.concatenate([ei[0], loops]), np.concatenate([ei[1], loops]))


def _tileize(dst_rel):
    spans = []
    i, n = 0, len(dst_rel)
    while i < n:
        j = min(i + 128, n)
        while j > i + 1 and dst_rel[j - 1] - dst_rel[i] > MAX_SPAN:
            j -= 1
        spans.append((i, j))
        i = j
    return spans


def _wrap16(flat):
    n = len(flat)
    cols = max((n + 15) // 16, 1)
    a = np.zeros((16, cols), np.int16)
    for i in range(n):
        a[i % 16, i // 16] = flat[i]
    return np.tile(a, (8, 1))


def bake_graph(src, dst):
    """Uniform-across-cores layout + per-core index data for one graph."""
    per_core = []
    for c in range(C):
        base = c * SHARD
        n_local = min(max(N_NODES - base, 0), SHARD)
        sel = (dst >= base) & (dst < base + n_local)
        s, d = src[sel], dst[sel] - base
        order = np.argsort(d, kind="stable")
        s, d = s[order], d[order]
        quads = []
        for q in range(NQUAD):
            m = (d >= q * 128) & (d < (q + 1) * 128)
            sq, dq = s[m], d[m] - q * 128
            quads.append([(sq[(sq & 1) == p], dq[(sq & 1) == p])
                          for p in (0, 1)])
        per_core.append(quads)

    tspans = [[[_tileize(per_core[c][q][p][1]) for p in (0, 1)]
               for q in range(NQUAD)] for c in range(C)]
    Tqc = [[max(len(tspans[c][q][p]) for c in range(C)) for p in (0, 1)]
           for q in range(NQUAD)]

    batches = []
    for q0 in range(0, NQUAD, QPB):
        qs = list(range(q0, min(q0 + QPB, NQUAD)))
        layout = [(q, p, Tqc[q][p]) for p in (0, 1) for q in qs]
        T0 = sum(Tqc[q][0] for q in qs)
        T1 = sum(Tqc[q][1] for q in qs)
        assert T0 + T1 <= MAXT, (T0, T1)
        batches.append(dict(quads=qs, layout=layout, T0=T0, T1=T1, T=T0 + T1))

    # per-core packed arrays
    packs = []
    for c in range(C):
        g0c, g1c, scatc, ohec, apgc = [], [], [], [], []
        for b in batches:
            tiles = []  # (quad, gidx[128], dstrel[k])
            for q, p, nt in b["layout"]:
                sq, dq = per_core[c][q][p]
                spans = tspans[c][q][p]
                for t in range(nt):
                    lo, hi = spans[t] if t < len(spans) else (len(sq), len(sq))
                    tiles.append((q, sq[lo:hi] >> 1, dq[lo:hi]))
            g0flat, g1flat = [], []
            for ti, (q, gi, dr) in enumerate(tiles):
                k = len(gi)
                arr = np.zeros(128, np.int16)
                arr[:k] = gi.astype(np.int16)
                (g0flat if ti < b["T0"] else g1flat).append(arr)
                sc = np.full(128, -1, np.int16)
                sc[:k] = ((ti % SCAT_SUB) * 128 + dr).astype(np.int16)
                scatc.append(sc)
                w8 = 0 if k == 0 else min(int(dr.min()) // 8, 12)
                oh = np.full(128, -1, np.int16)
                oh[:k] = (ti * 32 + (dr - 8 * w8)).astype(np.int16)
                assert k == 0 or int(dr.max()) - 8 * w8 < 32
                ohec.append(oh)
                blk = 16 * q + w8
                apgc.append(np.arange(blk, blk + 4, dtype=np.int16))
            g0c.append(_wrap16(np.concatenate(g0flat) if g0flat else
                               np.zeros(0, np.int16)))
            g1c.append(_wrap16(np.concatenate(g1flat) if g1flat else
                               np.zeros(0, np.int16)))
            apgc.append(None)  # marker: batch boundary handled below
        # apg packed per batch with wrap16
        apg_batches = []
        pos = 0
        apg_flat = [a for a in apgc if a is not None]
        for b in batches:
            chunk = np.concatenate(apg_flat[pos:pos + b["T"]])
            apg_batches.append(_wrap16(chunk))
            pos += b["T"]
        packs.append(dict(
            g0=np.concatenate(g0c, axis=1),
            g1=np.concatenate(g1c, axis=1),
            scat=np.stack(scatc, axis=1).reshape(128, -1) if False else
                 np.array(scatc).transpose(1, 0).astype(np.int16),
            ohe=np.array(ohec).transpose(1, 0).astype(np.int16),
            apg=np.concatenate(apg_batches, axis=1),
        ))
    return dict(batches=batches, packs=packs)


# ------------------------------------------------------------- device program
def build_program(bkx, bky):
    import concourse.bacc as bacc
    import concourse.mybir as mybir
    import concourse.tile as tile
    from concourse.masks import make_identity

    bf16, f32, i16 = mybir.dt.bfloat16, mybir.dt.float32, mybir.dt.int16
    AF = mybir.ActivationFunctionType
    ALU = mybir.AluOpType
    AX = mybir.AxisListType

    nc = bacc.Bacc("TRN2", target_bir_lowering=False, debug=False,
                   num_devices=C)

    xT = nc.dram_tensor("xT", [128, SHARD], bf16, kind="ExternalInput")
    wp_ = {}
    for nm in ["Wx1", "Wx2", "Wy1", "Wy2"]:
        wp_[nm] = nc.dram_tensor(nm, [128, 128], bf16, kind="ExternalInput")
    for nm in ["ax1", "ax2", "ay1", "ay2"]:
        wp_[nm] = nc.dram_tensor(nm, [128, 2], bf16, kind="ExternalInput")
    for nm in ["bx1", "bx2", "by1", "by2", "bfc"]:
        wp_[nm] = nc.dram_tensor(nm, [1, 128], f32, kind="ExternalInput")
    wp_["Wfc"] = nc.dram_tensor("Wfc", [256, 128], bf16, kind="ExternalInput")
    wp_["Wout"] = nc.dram_tensor("Wout", [128, 64], bf16, kind="ExternalInput")
    wp_["bout"] = nc.dram_tensor("bout", [64, 1], f32, kind="ExternalInput")

    def meta_tensors(tag, bk):
        T_tot = sum(b["T"] for b in bk["batches"])
        g0w = sum(8 * b["T0"] for b in bk["batches"])
        g1w = sum(8 * b["T1"] for b in bk["batches"])
        apw = sum(max((4 * b["T"] + 15) // 16, 1) for b in bk["batches"])
        return dict(
            g0=nc.dram_tensor(f"g0{tag}", [128, g0w], i16, kind="ExternalInput"),
            g1=nc.dram_tensor(f"g1{tag}", [128, g1w], i16, kind="ExternalInput"),
            scat=nc.dram_tensor(f"sc{tag}", [128, T_tot], i16, kind="ExternalInput"),
            ohe=nc.dram_tensor(f"oh{tag}", [128, T_tot], i16, kind="ExternalInput"),
            apg=nc.dram_tensor(f"ap{tag}", [128, apw], i16, kind="ExternalInput"),
        )

    mx = meta_tensors("x", bkx)
    my = meta_tensors("y", bky)
    oidx = nc.dram_tensor("oidx", [128, OUT_TILES * 8], i16, kind="ExternalInput")
    out_rows = nc.dram_tensor("out_rows", [OUT_TILES * 128, 128], f32,
                              kind="ExternalOutput")

    tables = {g: nc.dram_tensor(f"table_{g}", [NPAD, ELEM], bf16,
                                addr_space="Shared") for g in ("x", "y")}
    ag_in = nc.dram_tensor("ag_in", [SHARD, ELEM], bf16)
    final_dram = nc.dram_tensor("final_dram", [SHARD, 128], f32)

    with tile.TileContext(nc) as tc:
        with tc.tile_pool(name="pp", bufs=1) as pp, \
             tc.tile_pool(name="wp", bufs=1) as wp, \
             tc.tile_pool(name="sp", bufs=2) as sp, \
             tc.tile_pool(name="mp", bufs=2) as mp, \
             tc.tile_pool(name="psq", bufs=2, space="PSUM") as psq, \
             tc.tile_pool(name="psd", bufs=2, space="PSUM") as psd:

            ident32 = pp.tile([128, 128], f32, tag="ident32")
            make_identity(nc, ident32[:])
            identbf = pp.tile([128, 128], bf16, tag="identbf")
            nc.vector.tensor_copy(identbf[:], ident32[:])
            ones1 = pp.tile([1, 128], f32, tag="ones1")
            nc.vector.memset(ones1[:], 1.0)
            onesT = pp.tile([128, MAXT + 2], bf16, tag="onesT")
            nc.vector.memset(onesT[:], 1.0)

            hin_x = pp.tile([128, SHARD], bf16, tag="hinx")
            hin_y = pp.tile([128, SHARD], bf16, tag="hiny")
            hinT = {"x": hin_x, "y": hin_y}
            nc.sync.dma_start(hinT["x"][:], xT[:])
            nc.sync.dma_start(hinT["y"][:], xT[:])
            hl_x = pp.tile([128, SHARD], bf16, tag="hlx")
            hl_y = pp.tile([128, SHARD], bf16, tag="hly")
            hlast = {"x": hl_x, "y": hl_y}

            def bcast_row(src_row_ap, tag):
                t = wp.tile([128, 128], f32, tag=tag)
                ps = psd.tile([128, 512], f32, tag="ps512")
                nc.tensor.matmul(ps[:, 0:128], ones1[:], src_row_ap,
                                 start=True, stop=True)
                nc.vector.tensor_copy(t[:], ps[:, 0:128])
                return t

            def gat_layer(g, Wnm, anm, bnm, bk, meta, last):
                W = wp.tile([128, 128], bf16, tag="W")
                nc.sync.dma_start(W[:], wp_[Wnm][:])
                a_sd = wp.tile([128, 2], bf16, tag="a_sd")
                nc.sync.dma_start(a_sd[:], wp_[anm][:])
                b_row = wp.tile([1, 128], f32, tag="b_row")
                nc.sync.dma_start(b_row[:], wp_[bnm][:])
                b_bc = bcast_row(b_row[0:1, :], "b_bc")

                hkT = wp.tile([128, SHARD], bf16, tag="hkT")
                for j0 in range(0, SHARD, 512):
                    w = min(512, SHARD - j0)
                    ps = psd.tile([128, 512], f32, tag="ps512")
                    nc.tensor.matmul(ps[:, 0:w], W[:], hinT[g][:, j0:j0 + w],
                                     start=True, stop=True)
                    nc.vector.tensor_copy(hkT[:, j0:j0 + w], ps[:, 0:w])

                ed_rep = wp.tile([128, SHARD], bf16, tag="ed_rep")
                for j0 in range(0, SHARD, 512):
                    w = min(512, SHARD - j0)
                    ps = psd.tile([128, 512], f32, tag="ps512")
                    nc.tensor.matmul(ps[0:1, 0:w], a_sd[:, 1:2],
                                     hkT[:, j0:j0 + w], start=True, stop=True)
                    esed = wp.tile([1, 512], f32, tag="esed")
                    nc.vector.tensor_copy(esed[:, 0:w], ps[0:1, 0:w])
                    ps2 = psd.tile([128, 512], f32, tag="ps512")
                    nc.tensor.matmul(ps2[:, 0:w], ones1[:], esed[0:1, 0:w],
                                     start=True, stop=True)
                    nc.vector.tensor_copy(ed_rep[:, j0:j0 + w], ps2[:, 0:w])

                # table build + allgather (7-quad chunks)
                for c0 in range(0, NQUAD, 7):
                    tb = wp.tile([128, 7, ELEM], bf16, tag="tb")
                    nc.vector.memset(tb[:], 0.0)
                    for j in range(7):
                        q = c0 + j
                        ps = psd.tile([128, 128], bf16, tag="psT")
                        nc.tensor.transpose(ps[:], hkT[:, 128 * q:128 * (q + 1)],
                                            identbf[:])
                        nc.vector.tensor_copy(tb[:, j, 0:128], ps[:])
                        ps2 = psd.tile([128, 2], f32, tag="psT")
                        nc.tensor.matmul(ps2[:], hkT[:, 128 * q:128 * (q + 1)],
                                         a_sd[:], start=True, stop=True)
                        nc.vector.tensor_copy(
                            tb[:].bitcast(f32)[:, j, ES_F32:ES_F32 + 2], ps2[:])
                    nc.vector.memset(tb[:, :, 128:129], 1.0)
                    nc.sync.dma_start(
                        ag_in[c0 * 128:(c0 + 7) * 128, :]
                        .rearrange("(t p) e -> p t e", p=128), tb[:])
                if DO_AG:
                    nc.gpsimd.collective_compute(
                        "AllGather", mybir.AluOpType.bypass,
                        replica_groups=[list(range(C))],
                        ins=[ag_in[:]], outs=[tables[g][:]])
                else:
                    nc.sync.dma_start(tables[g][0:SHARD, :], ag_in[:])

                tblv = tables[g][:].rearrange("(n two) e -> n (two e)", two=2)
                tbl_even = tblv[:, 0:ELEM]
                tbl_odd = tblv[:, ELEM:2 * ELEM]

                hT_out = hlast[g] if last else hinT[g]

                g0o = g1o = apo = tpos = 0
                for b in (bk["batches"] if DO_EDGES else []):
                    T0, T1, T = b["T0"], b["T1"], b["T"]
                    g0w, g1w = 8 * T0, 8 * T1
                    apw = max((4 * T + 15) // 16, 1)
                    slab = sp.tile([128, MAXT, ELEM], bf16, tag="slab")
                    idx0 = mp.tile([128, 8 * MAXT], i16, tag="idx0")
                    idx1 = mp.tile([128, 8 * MAXT], i16, tag="idx1")
                    scat_t = mp.tile([128, MAXT + 2], i16, tag="scat_t")
                    ohe_t = mp.tile([128, MAXT + 2], i16, tag="ohe_t")
                    apg_t = mp.tile([128, (4 * MAXT) // 16 + 1], i16, tag="apg_t")
                    nc.vector.memset(scat_t[:], -1)
                    nc.vector.memset(ohe_t[:], -1)
                    if T0:
                        nc.sync.dma_start(idx0[:, 0:g0w],
                                          meta["g0"][:, g0o:g0o + g0w])
                        for c0 in range(0, T0, 8):
                            cw = min(8, T0 - c0)
                            nc.gpsimd.dma_gather(
                                out_ap=slab[:, c0:c0 + cw, :], in_ap=tbl_even,
                                idxs_ap=idx0[:, 8 * c0:8 * (c0 + cw)],
                                num_idxs=cw * 128,
                                num_idxs_reg=cw * 128, elem_size=ELEM,
                                elem_step=2 * ELEM)
                    if T1:
                        nc.sync.dma_start(idx1[:, 0:g1w],
                                          meta["g1"][:, g1o:g1o + g1w])
                        for c0 in range(0, T1, 8):
                            cw = min(8, T1 - c0)
                            nc.gpsimd.dma_gather(
                                out_ap=slab[:, T0 + c0:T0 + c0 + cw, :],
                                in_ap=tbl_odd,
                                idxs_ap=idx1[:, 8 * c0:8 * (c0 + cw)],
                                num_idxs=cw * 128,
                                num_idxs_reg=cw * 128, elem_size=ELEM,
                                elem_step=2 * ELEM)
                    g0o += g0w
                    g1o += g1w
                    nc.sync.dma_start(scat_t[:, 0:T], meta["scat"][:, tpos:tpos + T])
                    nc.sync.dma_start(ohe_t[:, 0:T], meta["ohe"][:, tpos:tpos + T])
                    nc.sync.dma_start(apg_t[:, 0:apw], meta["apg"][:, apo:apo + apw])
                    apo += apw

                    ed_sel = mp.tile([128, MAXT * 4, 8], bf16, tag="ed_sel")
                    nc.gpsimd.ap_gather(
                        out_ap=ed_sel[:, 0:4 * T, :],
                        in_ap=ed_rep[:].rearrange("p (b e) -> p b e", e=8),
                        idxs_ap=apg_t[:, 0:apw],
                        channels=128, num_elems=SHARD // 8, d=8, num_idxs=4 * T)
                    ohE = mp.tile([128, MAXT * 32], bf16, tag="ohE")
                    Te = T + (T % 2)
                    nc.gpsimd.local_scatter(
                        out_ap=ohE[:, 0:T * 32], data_ap=onesT[:, 0:Te],
                        idxs_ap=ohe_t[:, 0:Te], channels=128,
                        num_elems=T * 32, num_idxs=Te)
                    prod = mp.tile([128, MAXT, 32], bf16, tag="prod")
                    nc.vector.tensor_tensor(
                        out=prod[:, 0:T, :],
                        in0=ohE[:, 0:T * 32].rearrange("p (t w) -> p t w", w=32),
                        in1=ed_sel[:, 0:4 * T, :].rearrange("p a b -> p (a b)")
                            .rearrange("p (t w) -> p t w", w=32),
                        op=ALU.mult)
                    ed_col = mp.tile([128, MAXT], f32, tag="ed_col")
                    nc.vector.tensor_reduce(out=ed_col[:, 0:T],
                                            in_=prod[:, 0:T, :],
                                            axis=AX.X, op=ALU.add)
                    z = mp.tile([128, MAXT], f32, tag="z")
                    nc.vector.tensor_tensor(
                        out=z[:, 0:T],
                        in0=slab[:].bitcast(f32)[:, 0:T, ES_F32:ES_F32 + 1],
                        in1=ed_col[:, 0:T], op=ALU.add)
                    z2 = mp.tile([128, MAXT], f32, tag="z2")
                    nc.vector.tensor_scalar_mul(z2[:, 0:T], z[:, 0:T], NEG_SLOPE)
                    nc.vector.tensor_tensor(out=z[:, 0:T], in0=z[:, 0:T],
                                            in1=z2[:, 0:T], op=ALU.max)
                    p_col = mp.tile([128, MAXT + 2], bf16, tag="p_col")
                    nc.scalar.activation(p_col[:, 0:T], z[:, 0:T], AF.Exp)
                    lhsT = mp.tile([128, MAXT * 128], bf16, tag="lhsT")
                    for s0 in range(0, T, SCAT_SUB):
                        sw = min(SCAT_SUB, T - s0)
                        swe = sw + (sw % 2)
                        nc.gpsimd.local_scatter(
                            out_ap=lhsT[:, s0 * 128:(s0 + sw) * 128],
                            data_ap=p_col[:, s0:s0 + swe],
                            idxs_ap=scat_t[:, s0:s0 + swe],
                            channels=128, num_elems=sw * 128, num_idxs=swe)

                    # per-quad aggregation + finalize
                    ti = 0
                    quad_tiles = {q: [] for q in b["quads"]}
                    for q, p, nt in b["layout"]:
                        quad_tiles[q].extend(range(ti, ti + nt))
                        ti += nt
                    for q in b["quads"]:
                        ps = psq.tile([128, 129], f32, tag="agg")
                        idxs = quad_tiles[q]
                        for k, t in enumerate(idxs):
                            nc.tensor.matmul(
                                ps[:], lhsT[:, t * 128:(t + 1) * 128],
                                slab[:, t, 0:129],
                                start=(k == 0), stop=(k == len(idxs) - 1))
                        den = mp.tile([128, 1], f32, tag="den")
                        nc.vector.tensor_scalar_add(den[:], ps[:, 128:129], 1e-30)
                        rec = mp.tile([128, 1], f32, tag="rec")
                        nc.vector.reciprocal(rec[:], den[:])
                        nm = mp.tile([128, 128], f32, tag="nm")
                        nc.vector.scalar_tensor_tensor(
                            out=nm[:], in0=ps[:, 0:128], scalar=rec[:],
                            in1=b_bc[:], op0=ALU.mult, op1=ALU.add)
                        hq = mp.tile([128, 128], f32, tag="hq")
                        nc.scalar.activation(hq[:], nm[:], AF.Relu)
                        pst = psd.tile([128, 128], f32, tag="psT")
                        nc.tensor.transpose(pst[:], hq[:], ident32[:])
                        nc.vector.tensor_copy(
                            hT_out[:, 128 * q:128 * (q + 1)], pst[:])
                    tpos += T
                if not DO_EDGES:
                    nc.vector.tensor_copy(hT_out[:], hkT[:])

            layer_list = [("x", "Wx1", "ax1", "bx1", bkx, mx, False),
                          ("x", "Wx2", "ax2", "bx2", bkx, mx, True),
                          ("y", "Wy1", "ay1", "by1", bky, my, False),
                          ("y", "Wy2", "ay2", "by2", bky, my, True)][:NLAYERS]
            for (g_, w_, a_, b_, bk_, m_, l_) in layer_list:
                gat_layer(g_, w_, a_, b_, bk_, m_, last=l_)
            if NLAYERS < 4:
                nc.vector.tensor_copy(hlast["x"][:], hinT["x"][:])
                nc.vector.tensor_copy(hlast["y"][:], hinT["y"][:])

            # final MLP
            Wfc_a = wp.tile([128, 128], bf16, tag="Wfc_a")
            nc.sync.dma_start(Wfc_a[:], wp_["Wfc"][0:128, :])
            Wfc_b = wp.tile([128, 128], bf16, tag="Wfc_b")
            nc.sync.dma_start(Wfc_b[:], wp_["Wfc"][128:256, :])
            bfc_row = wp.tile([1, 128], f32, tag="bfc_row")
            nc.sync.dma_start(bfc_row[:], wp_["bfc"][:])
            bfc_bc = bcast_row(bfc_row[0:1, :], "bfc_bc")
            Wout_t = wp.tile([128, 64], bf16, tag="Wout_t")
            nc.sync.dma_start(Wout_t[:], wp_["Wout"][:])
            bout_c = wp.tile([64, 1], f32, tag="bout_c")
            nc.sync.dma_start(bout_c[:], wp_["bout"][:])

            hfcT = wp.tile([128, SHARD], bf16, tag="hfcT")
            for j0 in range(0, SHARD, 512):
                w = min(512, SHARD - j0)
                ps = psd.tile([128, 512], f32, tag="ps512")
                nc.tensor.matmul(ps[:, 0:w], Wfc_a[:],
                                 hlast["x"][:, j0:j0 + w],
                                 start=True, stop=False)
                nc.tensor.matmul(ps[:, 0:w], Wfc_b[:],
                                 hlast["y"][:, j0:j0 + w],
                                 start=False, stop=True)
                t1 = mp.tile([128, 512], f32, tag="fc_t1")
                for k in range(w // 128):
                    nc.vector.tensor_tensor(
                        out=t1[:, 128 * k:128 * (k + 1)],
                        in0=ps[:, 128 * k:128 * (k + 1)], in1=bfc_bc[:],
                        op=ALU.add)
                nc.scalar.activation(hfcT[:, j0:j0 + w], t1[:, 0:w], AF.Relu)

            for q in range(NQUAD):
                ps = psd.tile([128, 512], f32, tag="ps512")
                nc.tensor.matmul(ps[0:64, 0:128], Wout_t[:],
                                 hfcT[:, 128 * q:128 * (q + 1)],
                                 start=True, stop=True)
                ob = mp.tile([64, 128], f32, tag="ob")
                nc.vector.tensor_scalar_add(ob[:], ps[0:64, 0:128], bout_c[:])
                ps3 = psd.tile([128, 128], f32, tag="psT")
                nc.tensor.transpose(ps3[:, 0:64], ob[:], ident32[0:64, 0:64])
                fin = mp.tile([128, 64], f32, tag="fin")
                nc.vector.tensor_copy(fin[:], ps3[:, 0:64])
                nc.sync.dma_start(final_dram[128 * q:128 * (q + 1), 0:64], fin[:])

            if DO_OG:
                oi = mp.tile([128, OUT_TILES * 8], i16, tag="oi")
                nc.sync.dma_start(oi[:], oidx[:])
                og = sp.tile([128, OUT_TILES, 256], bf16, tag="og")
                for c0 in range(0, OUT_TILES, 8):
                    nc.gpsimd.dma_gather(
                        out_ap=og[:, c0:c0 + 8, :],
                        in_ap=final_dram[:].bitcast(bf16),
                        idxs_ap=oi[:, 8 * c0:8 * (c0 + 8)],
                        num_idxs=1024, num_idxs_reg=1024,
                        elem_size=256)
                nc.sync.dma_start(
                    out_rows[:].bitcast(bf16)
                    .rearrange("(t p) e -> p t e", p=128), og[:])
            else:
                nc.sync.dma_start(out_rows[:], final_dram[0:OUT_TILES * 128, :])

    nc.finalize()
    return nc


_PROGRAM_CACHE = {}
LAST_EXEC_NS = None


def kernel(**inputs):
    from concourse.bass_utils import run_bass_kernel_spmd
    import ml_dtypes

    bf = ml_dtypes.bfloat16
    x = np.asarray(inputs["x"], np.float32)
    eix = np.asarray(inputs["edge_index_x"], np.int64)
    eiy = np.asarray(inputs["edge_index_y"], np.int64)
    indices = np.asarray(inputs["indices"], np.int64)

    key = (eix.tobytes(), eiy.tobytes())
    import hashlib
    key = hashlib.sha1(key[0] + key[1]).hexdigest()
    if key in _PROGRAM_CACHE:
        nc, bkx, bky = _PROGRAM_CACHE[key]
    else:
        sx, dx = _with_loops(eix)
        sy, dy = _with_loops(eiy)
        bkx = bake_graph(sx, dx)
        bky = bake_graph(sy, dy)
        nc = build_program(bkx, bky)
        _PROGRAM_CACHE[key] = (nc, bkx, bky)

    xT = np.zeros((128, NPAD), np.float32)
    xT[:, :N_NODES] = x.T

    # output gather baking
    out_meta = []
    for c in range(C):
        sel = np.where((indices >= c * SHARD) & (indices < (c + 1) * SHARD))[0]
        loc = (indices[sel] - c * SHARD).astype(np.int16)
        assert len(sel) <= OUT_TILES * 128
        flat = np.zeros(OUT_TILES * 128, np.int16)
        flat[:len(loc)] = loc
        out_meta.append((sel, _wrap16(flat)))

    def cm(a, dt=None):
        a = np.asarray(a)
        return np.ascontiguousarray(a.astype(dt) if dt is not None else a)

    in_maps = []
    for c in range(C):
        m = dict(
            xT=cm(xT[:, c * SHARD:(c + 1) * SHARD].astype(bf)),
            Wx1=cm(inputs["Wx1"], bf), Wx2=cm(inputs["Wx2"], bf),
            Wy1=cm(inputs["Wy1"], bf), Wy2=cm(inputs["Wy2"], bf),
            ax1=cm(np.stack([inputs["asx1"], inputs["adx1"]], 1), bf),
            ax2=cm(np.stack([inputs["asx2"], inputs["adx2"]], 1), bf),
            ay1=cm(np.stack([inputs["asy1"], inputs["ady1"]], 1), bf),
            ay2=cm(np.stack([inputs["asy2"], inputs["ady2"]], 1), bf),
            bx1=cm(inputs["bx1"][None, :], np.float32),
            bx2=cm(inputs["bx2"][None, :], np.float32),
            by1=cm(inputs["by1"][None, :], np.float32),
            by2=cm(inputs["by2"][None, :], np.float32),
            bfc=cm(inputs["bfc"][None, :], np.float32),
            Wfc=cm(inputs["Wfc"], bf),
            Wout=cm(inputs["Wout"], bf),
            bout=cm(np.asarray(inputs["bout"])[:, None], np.float32),
            oidx=cm(out_meta[c][1]),
        )
        for tag, bk in (("x", bkx), ("y", bky)):
            pk = bk["packs"][c]
            m[f"g0{tag}"] = cm(pk["g0"])
            m[f"g1{tag}"] = cm(pk["g1"])
            m[f"sc{tag}"] = cm(pk["scat"])
            m[f"oh{tag}"] = cm(pk["ohe"])
            m[f"ap{tag}"] = cm(pk["apg"])
        in_maps.append(m)

    import time as _time
    t0 = _time.time()
    res = run_bass_kernel_spmd(nc, in_maps, list(range(C)))
    t1 = _time.time()
    global LAST_EXEC_NS
    if os.environ.get("GAT_BENCH", "0") == "1":
        t2 = _time.time()
        res = run_bass_kernel_spmd(nc, in_maps, list(range(C)))
        t3 = _time.time()
        LAST_EXEC_NS = int((t3 - t2) * 1e9)
        print(f"bench: first call {t1 - t0:.2f}s, second {t3 - t2:.2f}s")

    out = np.zeros((len(indices), 64), np.float32)
    for c in range(C):
        sel, _ = out_meta[c]
        rows = np.asarray(res.results[c]["out_rows"])
        out[sel] = rows[:len(sel), 0:64]
    return out
